# revision 3
# baseline (speedup 1.0000x reference)
"""ABCDense Trainium2 kernel v4: fp8 DoubleRow GEMM, PE fp8 transposes.

Math (per-column-normalized fp8 folding, offline rel err 0.51%):
    abar = (a0+a1+a2)/3;  r_e = a_e/abar
    W8   = fp8e4( sum_e sign(k_e) * r_e[None,:] )     values near {+-1,+-3}
    out  = (sign(x) @ W8) * (beta_raw[:,None] * (abar/D)[None,:])

Trace-driven structure (v2/v3 post-mortems):
  - x f32 on the two HWDGE rings (SWDGE cast-DMA measured ~100GB/s: dead).
  - sign: ACT f32 -> fp8 directly; PE transposes fp8 (1 cyc/row) into fp8
    PSUM; evac group0 on ACT (Copy), group1 on DVE (tensor_copy).
  - alpha math in [128,8] partition-parallel layout (reciprocal 109ns vs
    7.8us row-wise), DRAM roundtrip + loads on ring B, gpsimd broadcasts.
  - k loads first on both rings; k-signs ACT(e0,e1)/DVE(e2 2-op); fold on
    DVE bf16 2x (526ns/op measured); MM pair-rotation (t%4) so early
    tiles do not gate on the last W8 chunk.
  - evac: h0 DVE STT (ps*beta)*dq; h1 ACT Copy*beta -> DVE dq-mult bf16.
"""

import numpy as np

import concourse.mybir as mybir
from concourse import bacc, tile
from concourse.bass_utils import run_bass_kernel_spmd
from concourse.masks import make_identity

F32 = mybir.dt.float32
BF16 = mybir.dt.bfloat16
FP8 = mybir.dt.float8e4
AF = mybir.ActivationFunctionType
ALU = mybir.AluOpType
DR = mybir.MatmulPerfMode.DoubleRow

N, D, U, E = 32768, 1024, 1024, 3
NCORES = 8
NS = N // NCORES
P = 128
DC = D // P
NT = NS // P
UH = 512
LOOK = 10                   # x load lead (tiles)
TL = 2                      # transpose lead (tiles)


def build_nc():
    nc = bacc.Bacc(
        "TRN2",
        target_bir_lowering=False,
        debug=False,
        num_devices=NCORES,
    )

    x_d = nc.dram_tensor("x", [NS, D], F32, kind="ExternalInput")
    ks = [nc.dram_tensor(f"k{e}", [D, U], F32, kind="ExternalInput") for e in range(E)]
    as_ = [nc.dram_tensor(f"a{e}", [1, U], F32, kind="ExternalInput") for e in range(E)]
    out_d = nc.dram_tensor("out", [NS, U], BF16, kind="ExternalOutput")
    r_d = [nc.dram_tensor(f"r{e}_scratch", [1, U], BF16, kind="Internal")
           for e in range(E)]
    dq_d = nc.dram_tensor("dq_scratch", [1, U], F32, kind="Internal")

    with tile.TileContext(nc) as tc:
        with (
            tc.tile_pool(name="const", bufs=1) as const,
            tc.tile_pool(name="kstage", bufs=6) as kpool,
            tc.tile_pool(name="sgn", bufs=4) as spool,
            tc.tile_pool(name="ftmp", bufs=2) as fpool,
            tc.tile_pool(name="xin", bufs=LOOK + 1) as xpool,
            tc.tile_pool(name="scr", bufs=2) as scrpool,
            tc.tile_pool(name="xq", bufs=TL + 2) as xqpool,
            tc.tile_pool(name="xqt", bufs=TL + 2) as xqtpool,
            tc.tile_pool(name="osb", bufs=3) as opool,
            tc.tile_pool(name="htmp", bufs=2) as hpool,
            tc.tile_pool(name="psT", bufs=2, space="PSUM") as psumT,
            tc.tile_pool(name="psM", bufs=6, space="PSUM") as psumM,
        ):
            rings = [nc.sync, nc.scalar]

            ident = const.tile([P, P], BF16)
            make_identity(nc, ident[:])
            beta_cols = const.tile([P, NT], F32)

            # ---- a loads as [1,U] rows, FIRST on ring A (contiguous) ----
            a_rows = []
            for e in range(E):
                a_f = const.tile([1, U], F32, tag=f"a_f{e}")
                nc.sync.dma_start(out=a_f[0:1, :], in_=as_[e][:, :])
                a_rows.append(a_f)

            # ---- first x tiles ----
            xts = {}

            def dma_x(t):
                x_t = xpool.tile([P, D], F32, tag="xt")
                rings[t % 2].dma_start(out=x_t[:], in_=x_d[t * P:(t + 1) * P, :])
                xts[t] = x_t

            dma_x(0)
            dma_x(1)

            # first k chunks ahead of the alpha roundtrip on the rings
            kes = {}

            def dma_k(c):
                for e in range(E):
                    ke = kpool.tile([P, U], F32, tag="ke")
                    rings[(c * E + e) % 2].dma_start(
                        out=ke[:], in_=ks[e][c * P:(c + 1) * P, :]
                    )
                    kes[(c, e)] = ke

            for c in range(3):
                dma_k(c)

            # ---- alpha math: row ops on DVE (adds + exact reciprocal +
            # dq), broadcasts on gpsimd, full-tile r_e via STT on DVE ----
            ab3 = const.tile([1, U], F32, tag="ab3")
            nc.vector.tensor_tensor(
                ab3[0:1, :], a_rows[0][0:1, :], a_rows[1][0:1, :], op=ALU.add
            )
            nc.vector.tensor_tensor(
                ab3[0:1, :], ab3[0:1, :], a_rows[2][0:1, :], op=ALU.add
            )
            rec_row = const.tile([1, U], F32, tag="rec_row")
            nc.vector.reciprocal(rec_row[0:1, :], ab3[0:1, :])
            dq_row = const.tile([1, U], F32, tag="dq_row")
            nc.vector.tensor_scalar(
                dq_row[0:1, :], ab3[0:1, :], 1.0 / (3.0 * D), None, op0=ALU.mult
            )
            a_bc = []
            for e in range(E):
                a_b = const.tile([P, U], F32, tag=f"a_bc{e}")
                nc.gpsimd.partition_broadcast(a_b[:], a_rows[e][0:1, :])
                a_bc.append(a_b)
            rec_bc = const.tile([P, U], F32, tag="rec_bc")
            nc.gpsimd.partition_broadcast(rec_bc[:], rec_row[0:1, :])
            dq_bc = const.tile([P, U], F32, tag="dq_bc")
            nc.gpsimd.partition_broadcast(dq_bc[:], dq_row[0:1, :])
            r_bc = []
            for e in range(E):
                r_b = const.tile([P, U], BF16, tag=f"r_bc{e}")
                nc.vector.scalar_tensor_tensor(
                    r_b[:], a_bc[e][:], 3.0, rec_bc[:], op0=ALU.mult, op1=ALU.mult
                )
                r_bc.append(r_b)

            # ---- remaining k chunk loads on both rings ----
            for c in range(3, DC):
                dma_k(c)

            # ---- beta (DVE STT f32 + scratch; measured 1.18us) ----
            def beta(t):
                scratch = scrpool.tile([P, D], F32, tag="scratch")
                nc.vector.scalar_tensor_tensor(
                    scratch[:], xts[t][:], -1.0, xts[t][:],
                    op0=ALU.mult, op1=ALU.max,
                    accum_out=beta_cols[:, t:t + 1],
                )

            # ---- steady-state helpers ----
            def sign_x(t):
                xq = xqpool.tile([P, D], BF16, tag="xq")
                nc.scalar.activation(xq[:], xts[t][:], AF.Sign)
                return xq

            xqTs = {}

            def transpose(t, xq):
                xqT = xqtpool.tile([P, DC, P], FP8, tag="xqT")
                for g in range(2):
                    psT = psumT.tile([P, 4, P], BF16, tag="psT")
                    for j in range(4):
                        c = 4 * g + j
                        nc.tensor.transpose(
                            psT[:, j, :], xq[:, c * P:(c + 1) * P], ident[:]
                        )
                    if g == 0:
                        nc.scalar.activation(xqT[:, 0:4, :], psT[:], AF.Copy)
                    else:
                        nc.vector.tensor_copy(xqT[:, 4:8, :], psT[:])
                xqTs[t] = xqT

            # ---- W8 fold + early sign/transpose interleaved on ACT ----
            W8 = const.tile([P, DC, U], FP8)
            for c in range(DC):
                sgns = []
                for e in range(E):
                    s = spool.tile([P, U], BF16, tag="sgn")
                    nc.scalar.activation(s[:], kes[(c, e)][:], AF.Sign)
                    sgns.append(s)
                if c < TL:
                    transpose(c, sign_x(c))
                if c + 2 < LOOK:
                    dma_x(c + 2)
                t0 = fpool.tile([P, U], BF16, tag="ft0")
                t1 = fpool.tile([P, U], BF16, tag="ft1")
                nc.vector.tensor_tensor(t0[:], sgns[0][:], r_bc[0][:], op=ALU.mult)
                nc.vector.tensor_tensor(t1[:], sgns[1][:], r_bc[1][:], op=ALU.mult)
                nc.vector.tensor_tensor(t0[:], t0[:], t1[:], op=ALU.add)
                nc.vector.tensor_tensor(t1[:], sgns[2][:], r_bc[2][:], op=ALU.mult)
                nc.vector.tensor_tensor(W8[:, c, :], t0[:], t1[:], op=ALU.add)

            # dq in bf16 for the h1 dq-multiply (one-time DVE copy, after folds)
            dq_bc_bf = const.tile([P, U], BF16, tag="dq_bc_bf")
            nc.vector.tensor_copy(dq_bc_bf[:], dq_bc[:])

            # betas for the first tiles, after the folds in the DVE queue
            beta(0)
            beta(1)

            # ---- main loop ----
            for t in range(NT):
                xqT = xqTs.pop(t)
                xts.pop(t)
                ps0 = psumM.tile([P, UH], F32, tag="ps")
                ps1 = psumM.tile([P, UH], F32, tag="ps")
                ps = [ps0, ps1]
                rot = t % 4
                for i in range(4):
                    cp = (rot + i) % 4
                    for h in range(2):
                        nc.tensor.matmul(
                            ps[h][:],
                            xqT[:, 2 * cp:2 * cp + 2, :],
                            W8[:, 2 * cp:2 * cp + 2, h * UH:(h + 1) * UH],
                            start=(i == 0), stop=(i == 3),
                            perf_mode=DR,
                        )
                osb = opool.tile([P, U], BF16, tag="osb")
                bcol = beta_cols[:, t:t + 1]
                nc.vector.scalar_tensor_tensor(
                    osb[:, 0:UH], ps0[:], bcol, dq_bc[:, 0:UH],
                    op0=ALU.mult, op1=ALU.mult,
                )
                htmp = hpool.tile([P, UH], BF16, tag="htmp")
                nc.scalar.activation(htmp[:], ps1[:], AF.Copy, scale=bcol)
                nc.vector.tensor_tensor(
                    osb[:, UH:U], htmp[:], dq_bc_bf[:, UH:U], op=ALU.mult
                )
                rings[(t + 1) % 2].dma_start(
                    out=out_d[t * P:(t + 1) * P, :], in_=osb[:]
                )
                if t + TL < NT:
                    transpose(t + TL, sign_x(t + TL))
                if t + 2 < NT:
                    beta(t + 2)
                if t + LOOK < NT:
                    dma_x(t + LOOK)

    nc.compile()
    return nc


_CACHE = {}


def _get_nc():
    if "nc" not in _CACHE:
        _CACHE["nc"] = build_nc()
    return _CACHE["nc"]


def make_in_maps(x, k0, k1, k2, a0, a1, a2):
    x = np.ascontiguousarray(x, dtype=np.float32)
    ks = [np.ascontiguousarray(k, dtype=np.float32) for k in (k0, k1, k2)]
    as_ = [np.ascontiguousarray(a, dtype=np.float32).reshape(1, U) for a in (a0, a1, a2)]
    in_maps = []
    for i in range(NCORES):
        shard = np.ascontiguousarray(x[i * NS:(i + 1) * NS])
        in_maps.append({
            "x": shard,
            **{f"k{e}": ks[e] for e in range(E)},
            **{f"a{e}": as_[e] for e in range(E)},
        })
    return in_maps


def run_sharded(x, k0, k1, k2, a0, a1, a2, trace=False, **kw):
    nc = _get_nc()
    in_maps = make_in_maps(x, k0, k1, k2, a0, a1, a2)
    res = run_bass_kernel_spmd(nc, in_maps, list(range(NCORES)), trace=trace, **kw)
    out = np.concatenate(
        [np.asarray(res.results[i]["out"]).astype(np.float32) for i in range(NCORES)],
        axis=0,
    )
    return out, res


def kernel(x, k0, k1, k2, a0, a1, a2):
    out, _ = run_sharded(x, k0, k1, k2, a0, a1, a2, trace=False)
    return out
